# revision 42
# baseline (speedup 1.0000x reference)
"""Trainium2 Bass kernel for nn_BaseSparseConn (gnn_message_passing).

Computes out = x @ conn + bias where conn is given in COO form
(rows = dst, cols = src of the transposed matrix):
    out.T[r, :] = sum_{e: rows[e]==r} values[e] * x[:, cols[e]]  + bias[r]

Strategy (8 NeuronCores, SPMD — one NEFF, per-core data):
  - Row-partition the output: core c owns output rows [c*12500, (c+1)*12500).
  - Per core, rows are processed in 98 blocks of 128 rows, in 14 groups of
    7 blocks.  A group's edges (avg ~14.3k) are fetched with dma_gather
    (SWDGE) from a zero-padded fp16 copy of x^T; each gather element is the
    real 128 B batch row (the upstream 256 B minimum is a transpose-only
    restriction, bypassed via _dma_gather_small).
  - dma_gather needs int16 indices, so the column space is covered by
    OVERLAPPING windows of 32768 rows (count chosen at runtime between 4
    and 5 by whichever yields fewer descriptors; the window overhang is
    split evenly between down-/up-shift zones).  Each edge is assignable
    to 1-2 windows; a host-side exact interval-scheduling rebalancer
    (Hall bound + EDF greedy) packs each (block, window) bucket to the
    minimal feasible capacity, minimizing descriptor padding (SWDGE
    desc-gen is the kernel's bottleneck at ~2.5-2.8 ns/descriptor,
    serialized across the Q7 cluster; ~94% of the kernel's span).
  - Scatter-add into the 128 output rows of a block is a one-hot matmul:
    ONE batched DVE tensor_tensor per group builds the fp8 one-hot
    M_eq[p, c, m] = (iota[m] == rows[p, c]) for every chunk-copy (row 255
    = padding, matches nothing; 0.0/1.0 are fp8-exact and the PE accepts
    fp8 lhsT with fp16 rhs), emitted one group ahead of the value
    multiplies so the DVE queue head never blocks on gather DMA.  Values
    are multiplied into the gathered data with one contiguous DVE op per
    window (each waits only on its own gather's transfer).  The PE
    accumulates psum[128 rows, 64 batch] += M_eq^T @ gathered across the
    block's chunk list; chunks on bucket boundaries appear in both
    neighbours' lists with non-member rows set to 255.
  - Bias rides on the PSUM->SBUF copy (scalar activation Identity with a
    per-partition bias operand); output DMAs issue from the scalar queue.
  - Pool depths are tuned sweet spots: gather 4 (deeper pools regress
    desc-gen itself via SDMA ring await_space backpressure: 4 bufs ->
    473us GPSIMD active, 5 -> ~543, 7 -> ~582), one-hot 3, bias 14
    (lives until the group's last psum drain).
"""

import numpy as np

# Problem constants (hardcoded per the harness contract)
B = 64
IN_F = 100000
OUT_F = 100000
N_CORES = 8

# Sharding / layout constants
ROWS_PER_CORE = OUT_F // N_CORES  # 12500
BLK = 128
GROUP = 7                         # blocks per gather group (98 = 14*7)
WIN_W = 32768                     # int16 gather index bound
WPAD = 6384                       # x^T zero padding (both ends)
XPAD = 128                        # x^T row stride (256 B, DGE requirement)


def _cdiv(a, b):
    return -(-a // b)


class Cfg:
    """Geometry shared between host-side data prep and the device program."""

    def __init__(self, in_f, out_f, batch, n_cores, rows_per_core, group,
                 n_win, win_stride, cap, wpad, xpad=128, blk=128):
        self.in_f = in_f
        self.out_f = out_f
        self.batch = batch
        self.n_cores = n_cores
        self.rows_per_core = rows_per_core
        self.blk = blk
        self.group = group
        self.n_win = n_win
        self.win_stride = win_stride
        self.wpad = wpad
        self.cap = cap                       # edges per (block, window) bucket
        self.xpad = xpad
        self.n_blocks = _cdiv(rows_per_core, blk)       # blocks per core
        assert self.n_blocks % group == 0, (self.n_blocks, group)
        self.n_groups = self.n_blocks // group
        self.ext_rows = in_f + 2 * wpad
        # per (group, window) gather section
        self.elems_sec = _cdiv(group * cap, 128) * 128
        self.chunks_sec = self.elems_sec // 128
        self.npart_w = self.elems_sec // 16   # idx free width per section
        self.slots_pg = n_win * self.chunks_sec
        # per-block chunk span (static): block j covers elements
        # [j*cap, (j+1)*cap) of each section
        self.chunk_lo = [(cap * j) // 128 for j in range(group)]
        self.chunk_hi = [(cap * (j + 1) - 1) // 128 for j in range(group)]
        self.span = [hi - lo + 1
                     for lo, hi in zip(self.chunk_lo, self.chunk_hi)]
        self.cpt = [n_win * s for s in self.span]   # chunk-copies per block
        self.cpt_max = max(self.cpt)
        self.rv_off = np.concatenate([[0], np.cumsum(self.cpt)]).astype(int)
        self.rv_w = int(self.rv_off[-1])      # rows free-dim per group
        self.out_rows = self.n_blocks * blk   # padded output rows per core


def _edge_stats(rows, cols, n_win, win_stride, wpad):
    """Per-core eligibility intervals + per-block Hall capacity bound."""
    rows = np.asarray(rows).astype(np.int64)
    cols = np.asarray(cols).astype(np.int64)
    n_blocks = _cdiv(ROWS_PER_CORE, BLK)
    per_core = []
    gmax = 1
    for c in range(N_CORES):
        e0, e1 = np.searchsorted(rows, [c * ROWS_PER_CORE,
                                        (c + 1) * ROWS_PER_CORE])
        r_loc = (rows[e0:e1] - c * ROWS_PER_CORE).astype(np.int64)
        col = cols[e0:e1]
        blk_id = r_loc // BLK
        nat = col // win_stride
        cmod = col % win_stride
        # window w covers cols [w*stride - wpad, w*stride - wpad + WIN_W)
        over = win_stride + wpad - WIN_W     # <= 0
        can_dn = (cmod < wpad) & (nat > 0)
        can_up = (cmod >= win_stride + over) & (nat < n_win - 1)
        lo_e = nat - can_dn
        dual = can_dn | can_up
        bounds = np.searchsorted(blk_id, np.arange(n_blocks + 1))
        blocks = []
        for bidx in range(n_blocks):
            s0, s1 = bounds[bidx], bounds[bidx + 1]
            sb = np.bincount(nat[s0:s1][~dual[s0:s1]], minlength=n_win)
            db = np.bincount(lo_e[s0:s1][dual[s0:s1]], minlength=n_win)
            tmin = 0
            for a in range(n_win):
                for b2 in range(a, n_win):
                    tot = sb[a:b2 + 1].sum() + db[a:b2].sum()
                    tmin = max(tmin, _cdiv(int(tot), b2 - a + 1))
            gmax = max(gmax, tmin)
            blocks.append((s0, s1, sb, db))
        per_core.append(dict(e0=e0, e1=e1, r_loc=r_loc, col=col,
                             lo_e=lo_e, dual=dual, blocks=blocks,
                             n_win=n_win))
    return per_core, gmax


def _finalize_windows(per_core, cap, n_win):
    """Greedy lowest-window-first (EDF) assignment at capacity `cap`."""
    for ed in per_core:
        lo_e, dual = ed["lo_e"], ed["dual"]
        win = lo_e.copy()
        for (s0, s1, sb, db) in ed["blocks"]:
            carry = 0
            for w in range(n_win):
                room = cap - int(sb[w]) - carry
                assert room >= 0, (cap, sb, db)
                take = min(int(db[w]), room)
                if db[w] > take:
                    idx = np.where(dual[s0:s1] & (lo_e[s0:s1] == w))[0]
                    win[s0 + idx[take:]] = w + 1
                carry = int(db[w]) - take
            assert carry == 0
        ed["win"] = win


def plan_windows(rows, cols):
    """Pick the window count (4 or 5) minimizing descriptors per group,
    with the window overhang split evenly between down- and up-shifts."""
    best = None
    for n_win in (4, 5):
        win_stride = _cdiv(IN_F, n_win)
        wpad = (WIN_W - win_stride) // 2
        stats, gmax = _edge_stats(rows, cols, n_win, win_stride, wpad)
        descs = n_win * _cdiv(GROUP * gmax, 128) * 128
        if best is None or descs < best[0]:
            best = (descs, n_win, win_stride, wpad, stats, gmax)
    descs, n_win, win_stride, wpad, stats, gmax = best
    _finalize_windows(stats, gmax, n_win)
    return n_win, win_stride, wpad, stats, gmax


def prep_host_data(cfg, x, values, bias, edges_per_core):
    """Lay out per-core inputs for the device program."""
    values = np.asarray(values, dtype=np.float32)
    bias = np.asarray(bias, dtype=np.float32)
    x = np.asarray(x, dtype=np.float32)

    # zero-padded fp16 x^T: ext row i = x[:, i - WPAD], zeros outside
    xp = np.zeros((cfg.ext_rows, cfg.xpad), dtype=np.float16)
    xp[cfg.wpad:cfg.wpad + cfg.in_f, :cfg.batch] = x.T.astype(np.float16)

    # iota row [0..127] per partition (stride-0 broadcast along chunk-copies)
    iota = np.tile(np.arange(128, dtype=np.float16), (128, 1))

    chunk_lo = np.asarray(cfg.chunk_lo)
    span = np.asarray(cfg.span)
    rv_off = np.asarray(cfg.rv_off)

    per_core = []
    for c, ed in enumerate(edges_per_core):
        col = ed["col"]
        win = ed["win"].astype(np.int64)
        val = values[ed["e0"]:ed["e1"]]
        r_loc = ed["r_loc"]
        blk_id = r_loc // cfg.blk
        g_s = blk_id // cfg.group
        j_s = blk_id % cfg.group
        row_s = r_loc - blk_id * cfg.blk

        key = (g_s * cfg.group + j_s) * cfg.n_win + win
        order = np.argsort(key, kind="stable")
        key_s = key[order]
        col_s = col[order]
        val_s = val[order]
        row_ss = row_s[order]
        g_o = g_s[order]
        j_o = j_s[order]
        w_o = win[order]

        nbuckets = cfg.n_groups * cfg.group * cfg.n_win
        counts = np.bincount(key_s, minlength=nbuckets)
        assert counts.max() <= cfg.cap, (counts.max(), cfg.cap)
        starts = np.concatenate([[0], np.cumsum(counts)[:-1]])
        q = np.arange(len(key_s)) - starts[key_s]

        # element index within the (g, w) gather section
        elem = j_o * cfg.cap + q
        part = elem % 128
        chunk = elem // 128
        idxv = col_s + cfg.wpad - w_o * cfg.win_stride
        assert idxv.min() >= 0 and idxv.max() < WIN_W, (idxv.min(),
                                                        idxv.max())

        # ---- gather index array, 8x replicated across 128 partitions.
        # Element i of section (g, w) lives at [i % 16, i // 16].
        # Pad elements repeat the bucket's last real index (HBM row-hit).
        lastidx = np.zeros((cfg.n_groups, cfg.group, cfg.n_win),
                           dtype=np.int16)
        lastidx[g_o, j_o, w_o] = idxv.astype(np.int16)
        pad_idx = np.repeat(
            lastidx.transpose(0, 2, 1).reshape(cfg.n_groups, cfg.n_win,
                                               cfg.group, 1),
            cfg.cap, axis=3).reshape(cfg.n_groups, cfg.n_win, -1)
        tail = cfg.elems_sec - cfg.group * cfg.cap
        if tail:
            pad_idx = np.concatenate(
                [pad_idx, np.repeat(pad_idx[:, :, -1:], tail, axis=2)],
                axis=2)
        idx16 = pad_idx.reshape(cfg.n_groups, cfg.n_win, cfg.npart_w, 16
                                ).transpose(0, 3, 1, 2).reshape(
                                    cfg.n_groups, 16,
                                    cfg.n_win * cfg.npart_w)
        idx16 = np.ascontiguousarray(idx16)
        idx16[g_o, elem % 16, w_o * cfg.npart_w + elem // 16] = \
            idxv.astype(np.int16)
        idx_full = np.tile(idx16, (1, 8, 1))   # replicate to 128 partitions

        # ---- rows array: (n_groups, 128, rv_w) fp16; 255 = no match
        rv = np.full((cfg.n_groups, 128, cfg.rv_w), 255.0, dtype=np.float16)
        c_idx = w_o * span[j_o] + (chunk - chunk_lo[j_o])
        rv[g_o, part, rv_off[j_o] + c_idx] = row_ss.astype(np.float16)

        # ---- values array: (n_groups, 128, slots_pg) fp16 in slot order
        vb = np.zeros((cfg.n_groups, 128, cfg.slots_pg), dtype=np.float16)
        slot_g = w_o * cfg.chunks_sec + chunk
        vb[g_o, part, slot_g] = val_s.astype(np.float16)

        # ---- bias array: (n_groups, 128, group) fp16, per-partition bias
        # operand for the PSUM->SBUF activation copy
        bias_arr = np.zeros((cfg.n_groups, 128, cfg.group), dtype=np.float32)
        gg, mm, jj = np.meshgrid(np.arange(cfg.n_groups), np.arange(128),
                                 np.arange(cfg.group), indexing="ij")
        grow = c * cfg.rows_per_core + (gg * cfg.group + jj) * cfg.blk + mm
        valid = grow < min((c + 1) * cfg.rows_per_core, cfg.out_f)
        bias_arr[valid] = bias[grow[valid]]
        bias_arr = bias_arr.astype(np.float16)

        per_core.append({
            "xp": xp,
            "iota": iota,
            "idx": idx_full,
            "rv": rv,
            "valsb": vb,
            "biasb": bias_arr,
        })
    return per_core


def _dma_gather_small(gp, out_ap, in_ap, idxs_ap, num_idxs, num_idxs_reg,
                      elem_size, elem_step, single_packet, queue_num):
    """dma_gather (non-transpose, DRAM src) allowing elem < 256 B.

    Upstream bass.dma_gather asserts elem_size_bytes % 256 == 0, which the
    ucode only needs for transpose mode (256 B xbar spray descriptors); the
    non-transpose Q7 kernel emits arbitrary packet_bytes.  Gathering 128 B
    (the real 64xfp16 batch row) instead of a 256 B padded row halves the
    SDMA data volume.  The source stride must still be a multiple of 256 B.
    """
    import concourse.mybir as mybir
    from concourse import ap_utils
    from concourse.bass import MemorySpace
    from concourse._compat import exact_div, round_up_to_multiple

    gp._assert_queue_num(queue_num)
    assert idxs_ap.dtype == mybir.dt.int16
    assert in_ap.dtype == out_ap.dtype
    assert in_ap.space == MemorySpace.DRAM
    assert idxs_ap.space == MemorySpace.SBUF
    assert out_ap.space == MemorySpace.SBUF
    assert ap_utils.ap_is_contiguous(in_ap.ap[1:])
    assert ap_utils.ap_is_contiguous(out_ap.ap[1:])
    assert ap_utils.ap_is_contiguous(idxs_ap.ap[1:])
    assert in_ap.ap[-1][1] == out_ap.ap[-1][1] == elem_size
    assert out_ap.ap[0][1] * out_ap.ap[1][1] == round_up_to_multiple(
        num_idxs, 128)
    assert in_ap.ap[0][0] == elem_step
    stride_bytes = elem_step * mybir.dt.size(in_ap.dtype)
    stride_bytes_256 = exact_div(stride_bytes, 256)
    assert stride_bytes_256 < 256
    _in_ap = gp.lower_ap_dma(in_ap, for_custom_bir_dma=True)
    _idxs_ap = gp.lower_ap(idxs_ap)
    _out_ap = gp.lower_ap(out_ap)
    return gp.add_instruction(
        mybir.InstDMAGatherAnt(
            name=gp.bass.get_next_instruction_name(),
            ins=[*_in_ap, _idxs_ap,
                 gp.lower_val_access(gp.to_reg(num_idxs_reg))],
            outs=[_out_ap],
            transpose=False,
            num_idxs=num_idxs,
            elem_size=elem_size,
            stride_bytes_256=stride_bytes_256,
            gen_mode=0,
            single_packet=single_packet,
            queue_num=queue_num,
            sbuf_tokens_per_rank=0,
            sbuf_free_dim_per_rank=0,
            sbuf_free_dim_pad_per_rank=0,
            sbuf_byte_offset=0,
        ))


def build_program(cfg, enable_asserts=False, debug=False):
    import concourse.bacc as bacc
    import concourse.mybir as mybir
    import concourse.tile as tile
    import concourse.bass as bass_mod

    f16 = mybir.dt.float16
    f32 = mybir.dt.float32
    f8 = mybir.dt.float8e4
    i16 = mybir.dt.int16

    nc = bacc.Bacc("TRN2", target_bir_lowering=False, debug=debug,
                   enable_asserts=enable_asserts, num_devices=cfg.n_cores,
                   num_swdge_queues=4)

    xp_d = nc.dram_tensor("xp", (cfg.ext_rows, cfg.xpad), f16,
                          kind="ExternalInput")
    iota_d = nc.dram_tensor("iota", (128, 128), f16, kind="ExternalInput")
    idx_d = nc.dram_tensor("idx", (cfg.n_groups, 128,
                                   cfg.n_win * cfg.npart_w),
                           i16, kind="ExternalInput")
    rv_d = nc.dram_tensor("rv", (cfg.n_groups, 128, cfg.rv_w), f16,
                          kind="ExternalInput")
    vals_d = nc.dram_tensor("valsb", (cfg.n_groups, 128, cfg.slots_pg), f16,
                            kind="ExternalInput")
    bias_d = nc.dram_tensor("biasb", (cfg.n_groups, 128, cfg.group), f16,
                            kind="ExternalInput")
    out_d = nc.dram_tensor("out_t", (cfg.out_rows, cfg.batch), f32,
                           kind="ExternalOutput")

    with tile.TileContext(nc, num_cores=cfg.n_cores) as tc:
        with (
            tc.tile_pool(name="const", bufs=1) as cp,
            tc.tile_pool(name="meta", bufs=4) as mp_meta,
            tc.tile_pool(name="biasp", bufs=14) as bp,
            tc.tile_pool(name="gath", bufs=4) as gp,
            tc.tile_pool(name="mtile", bufs=3) as mp,
            tc.tile_pool(name="ostage", bufs=4) as op,
            tc.tile_pool(name="ps", bufs=8, space="PSUM") as pp,
        ):
            # hoist the constant num_idxs register: one MOVE instead of
            # one per gather on the GPSIMD queue
            nidx_reg = nc.gpsimd.to_reg(cfg.elems_sec)
            iota_t = cp.tile([128, 128], f16)

            def issue_group(g):
                """Meta DMAs + SWDGE gathers for group g."""
                h = {}
                idx_t = mp_meta.tile([128, cfg.n_win * cfg.npart_w], i16,
                                     tag="idx", name="idx_t")
                nc.sync.dma_start(out=idx_t[:], in_=idx_d[g])
                h["rv"] = mp_meta.tile([128, cfg.rv_w], f16, tag="rv",
                                       name="rv_t")
                nc.sync.dma_start(out=h["rv"][:], in_=rv_d[g])
                h["vals"] = mp_meta.tile([128, cfg.slots_pg], f16,
                                         tag="vals", name="vals_t")
                nc.sync.dma_start(out=h["vals"][:], in_=vals_d[g])
                # bias lives until the group's LAST psum drain — deep pool
                # so its reuse never blocks the sync queue head
                h["bias"] = bp.tile([128, cfg.group], f16,
                                    tag="bias", name="bias_t")
                nc.sync.dma_start(out=h["bias"][:], in_=bias_d[g])

                h["gath"] = gp.tile([128, cfg.slots_pg, cfg.batch], f16,
                                    tag="g", name="gath_t")
                for w in range(cfg.n_win):
                    lo = w * cfg.win_stride
                    _dma_gather_small(
                        nc.gpsimd,
                        out_ap=h["gath"][:, w * cfg.chunks_sec:
                                         (w + 1) * cfg.chunks_sec, :],
                        in_ap=xp_d[lo:lo + WIN_W, 0:cfg.batch],
                        idxs_ap=idx_t[:, w * cfg.npart_w:
                                      (w + 1) * cfg.npart_w],
                        num_idxs=cfg.elems_sec,
                        num_idxs_reg=nidx_reg,
                        elem_size=cfg.batch,
                        elem_step=cfg.xpad,
                        # one packet per descriptor: a coalesced stream of
                        # >64 descriptors/engine aborts the SDMA engine
                        single_packet=False,
                        # rotate rings so all 4 carry equal traffic
                        queue_num=(g * cfg.n_win + w) % 4,
                    )
                return h

            def issue_meqs(h):
                """The group's one-hot build — ONE batched DVE op covering
                every chunk-copy of every block, emitted one group ahead of
                the value-multiply so the DVE queue head never blocks on
                the gather DMA."""
                # M_eq[p, c, m] = (iota[p, m] == rows[p, c])
                # fp8 one-hot: 0.0/1.0 are exact, halves the SBUF
                # footprint so two 14-block groups fit in flight
                meq = mp.tile([128, cfg.rv_w, 128], f8, tag="meq",
                              name="meq_t")
                r0 = h["rv"][:, 0:1]
                rows_bcast = bass_mod.AP(
                    r0.tensor, r0.offset,
                    [r0.ap[0], [1, cfg.rv_w], [0, 128]])
                i0 = iota_t[:]
                iota_rep = bass_mod.AP(
                    i0.tensor, i0.offset,
                    [i0.ap[0], [0, cfg.rv_w], [1, 128]])
                nc.vector.tensor_tensor(
                    out=meq[:], in0=iota_rep, in1=rows_bcast,
                    op=mybir.AluOpType.is_equal)
                return meq

            def process_group(g, h, meq):
                gath = h["gath"]
                # gath[p, s, :] *= vals[p, s] in place — one contiguous DVE
                # op PER WINDOW, so each quarter only waits on its own
                # gather's DMA (overlaps the tail group's mult with its
                # remaining transfers)
                for w in range(cfg.n_win):
                    g0 = gath[:, w * cfg.chunks_sec, :]
                    gsec = bass_mod.AP(
                        g0.tensor, g0.offset,
                        [g0.ap[0], [cfg.batch, cfg.chunks_sec],
                         [1, cfg.batch]])
                    v0 = h["vals"][:, w * cfg.chunks_sec:
                                   w * cfg.chunks_sec + 1]
                    vals_bcast = bass_mod.AP(
                        v0.tensor, v0.offset,
                        [v0.ap[0], [1, cfg.chunks_sec], [0, cfg.batch]])
                    nc.vector.tensor_tensor(
                        out=gsec, in0=gsec, in1=vals_bcast,
                        op=mybir.AluOpType.mult)

                for j in range(cfg.group):
                    b = g * cfg.group + j
                    ps = pp.tile([128, cfg.batch], f32, tag="ps")
                    cc = 0
                    for w in range(cfg.n_win):
                        for ch in range(cfg.chunk_lo[j],
                                        cfg.chunk_hi[j] + 1):
                            slot = w * cfg.chunks_sec + ch
                            nc.tensor.matmul(
                                out=ps[:],
                                lhsT=meq[:, cfg.rv_off[j] + cc, :],
                                rhs=gath[:, slot, :],
                                start=(cc == 0),
                                stop=(cc == cfg.cpt[j] - 1),
                            )
                            cc += 1
                    assert cc == cfg.cpt[j]
                    o_t = op.tile([128, cfg.batch], f32, tag="o")
                    # PSUM->SBUF copy with bias folded in (per-partition)
                    nc.scalar.activation(
                        out=o_t[:], in_=ps[:],
                        func=mybir.ActivationFunctionType.Identity,
                        bias=h["bias"][:, j:j + 1])
                    # issue from the (idle) scalar queue so out DMAs never
                    # block meta loads on the sync queue
                    nc.scalar.dma_start(
                        out=out_d[b * cfg.blk:(b + 1) * cfg.blk, :],
                        in_=o_t[:],
                    )

            cur = issue_group(0)
            # iota load sits behind group 0's meta so the first gather's
            # index DMA is at the head of the sync queue
            nc.sync.dma_start(out=iota_t[:], in_=iota_d[:, :])
            cur_meqs = issue_meqs(cur)
            for g in range(cfg.n_groups):
                if g + 1 < cfg.n_groups:
                    nxt = issue_group(g + 1)
                    nxt_meqs = issue_meqs(nxt)
                else:
                    nxt = nxt_meqs = None
                process_group(g, cur, cur_meqs)
                cur, cur_meqs = nxt, nxt_meqs

    nc.compile()
    return nc


LAST_RESULT = None  # BassKernelResults of the most recent kernel() call


def kernel(x, values, bias, rows, cols):
    global LAST_RESULT
    from concourse.bass_utils import run_bass_kernel_spmd

    n_win, win_stride, wpad, edges_per_core, cap = plan_windows(rows,
                                                                      cols)
    cfg = Cfg(IN_F, OUT_F, B, N_CORES, ROWS_PER_CORE, GROUP, n_win,
              win_stride, cap, wpad, xpad=XPAD, blk=BLK)

    per_core = prep_host_data(cfg, x, values, bias, edges_per_core)
    nc = build_program(cfg)
    res = run_bass_kernel_spmd(nc, per_core, core_ids=list(range(N_CORES)))
    LAST_RESULT = res

    parts = [res.results[c]["out_t"][:ROWS_PER_CORE] for c in range(N_CORES)]
    out_t = np.concatenate(parts, axis=0)       # (OUT_F, B) f32
    return np.ascontiguousarray(out_t.T)        # (B, OUT_F) f32
